# revision 22
# baseline (speedup 1.0000x reference)
"""Trainium2 Bass kernel for nn_NoSoftmaxGPT2Model (4-layer GPT2, no softmax).

Strategy: the missing softmax makes attention linear, so (Q K^T) V is
reassociated to Q (K^T V) -- K^T V is only [64, 64] per head. This kills the
S x S attention entirely and makes every op except that contraction
token-local. We shard the 2048-token sequence across 8 NeuronCores (256
tokens each), replicate the weights, and per layer AllReduce only the tiny
[12, 64, 64] K^T V partial sums (196 KB).

On-chip layout: activations live in SBUF transposed, [feature_part, token_free]
(T-layout). LayerNorm stats (per-token sums over features = partition
reduction) are computed with ones-vector matmuls on the PE in f32r (1 cyc/row
at free>=256), broadcast to all 128 partitions with a rank-1 ones matmul, and
the rsqrt/normalize chain runs full-width on vector+scalar. LN gains are
folded into the following weight matrices on the host.

Scheduling: all weight DMAs are emitted up-front in consumption order on one
sync queue with deep tile-pool rings, so the HBM stream runs continuously --
including through the per-layer AllReduce windows. The Q projection is emitted
between the AllReduce trigger and its first consumer so the PE has work during
the collective. PSUM is split into a 6-bank ring (KV / MLP accumulators) and a
2-bank ring (everything else), letting the MLP hold its 6 output accumulators
across all 24 ff-tiles while z-tiles rotate through 2 banks.

kernel(**inputs) takes the full unsharded inputs and returns the full
[1, 2048, 768] output.
"""

import os
from contextlib import ExitStack

import numpy as np
import ml_dtypes

import jax
from jax.sharding import Mesh, PartitionSpec, NamedSharding

import concourse.bass as bass
import concourse.bacc as bacc
import concourse.mybir as mybir
import concourse.tile as tile
from concourse.tile import add_dep_helper
from concourse import bass2jax

from jax.experimental.shard_map import shard_map

N_CORES = 8
L, S, E, H, FF = 4, 2048, 768, 12, 3072
DH = E // H  # 64
T = S // N_CORES  # 256 tokens per core
KT = E // 128  # 6 feature tiles
FT = FF // 128  # 24 ff tiles
EPS = 1e-5

F32 = mybir.dt.float32
F32R = mybir.dt.float32r
BF16 = mybir.dt.bfloat16
FP8 = mybir.dt.float8e4
DR = mybir.MatmulPerfMode.DoubleRow
AF = mybir.ActivationFunctionType
AO = mybir.AluOpType


def _r(ap):
    """f32 -> f32r view for PE ops where reduced precision is fine."""
    return ap.bitcast(F32R)


def build_model(reps=1, n_layers=L, collective=True):
    dtw = BF16
    nc = bacc.Bacc(
        "TRN2", target_bir_lowering=False, debug=False, num_devices=N_CORES
    )

    # x0 arrives host-pre-transposed to T-layout [E, T]
    x0_d = nc.dram_tensor("x0", [E, T], F32, kind="ExternalInput").ap()
    # attention weights concatenated on the last axis: [wq | wkv | wo]
    watt_d = nc.dram_tensor("watt", [L, E, 4 * E], dtw, kind="ExternalInput").ap()
    wq_d = watt_d[:, :, 0:E]
    wkv_d = watt_d[:, :, E : 3 * E]
    wo_d = watt_d[:, :, 3 * E : 4 * E]
    # w1 packed per ff-tile for DoubleRow: w1p[l,j,p,s,ko,c] = W1q[l,(2s+ko)*128+p, j*128+c]
    w1_d = nc.dram_tensor("w1", [L, FT, 128, 3, 2, 128], FP8, kind="ExternalInput").ap()
    # w2 packed per ff-pair: w2p[l,t,p,ko,c] = W2q[l,(2t+ko)*128+p, c]
    w2_d = nc.dram_tensor("w2", [L, FT // 2, 128, 2, E], FP8, kind="ExternalInput").ap()
    # small f32 aux blob: s1[L,FF] | b1[L,FF] | s2[L,E] | bq[L,E] | bo[L,E] | b2[L,E] | lnfg[E] | lnfb[E]
    AUX_SZ = 2 * L * FF + 4 * L * E + 2 * E
    aux_d = nc.dram_tensor("aux", [AUX_SZ], F32, kind="ExternalInput").ap()
    _o = [0]

    def _aux(n):
        a = aux_d[_o[0] : _o[0] + n]
        _o[0] += n
        return a

    s1_d = _aux(L * FF).rearrange("(l f) -> l f", l=L)
    b1_d = _aux(L * FF).rearrange("(l f) -> l f", l=L)
    s2_d = _aux(L * E).rearrange("(l f) -> l f", l=L)
    bq_d = _aux(L * E).rearrange("(l f) -> l f", l=L)
    bo_d = _aux(L * E).rearrange("(l f) -> l f", l=L)
    b2_d = _aux(L * E).rearrange("(l f) -> l f", l=L)
    lnfg_d = _aux(E)
    lnfb_d = _aux(E)
    bkv_d = nc.dram_tensor("bkv", [L, 2 * E], dtw, kind="ExternalInput").ap()
    # output leaves in T-layout [E, T]; host transposes back
    out_d = nc.dram_tensor("out", [E, T], F32, kind="ExternalOutput").ap()

    with tile.TileContext(nc) as tc, ExitStack() as ctx:
        const = ctx.enter_context(tc.tile_pool(name="const", bufs=1))
        wpool = ctx.enter_context(tc.tile_pool(name="wpool", bufs=1))
        apool = ctx.enter_context(tc.tile_pool(name="apool", bufs=1))
        ps = ctx.enter_context(tc.tile_pool(name="ps", bufs=1, space="PSUM"))
        dram = ctx.enter_context(tc.tile_pool(name="dram", bufs=1, space="DRAM"))

        _prev_dma = [None]

        def sdma(dst, src):
            """sync-queue DMA with forced emission-order enqueue (prevents
            scheduler-reordered slot-wait deadlocks in the shared FIFO)."""
            inst = nc.sync.dma_start(dst, src)
            if _prev_dma[0] is not None:
                add_dep_helper(inst.ins, _prev_dma[0].ins, sync=False, reason="dma order")
            _prev_dma[0] = inst
            return inst

        def psA(name):
            return ps.tile([128, 512], F32, tag="A", bufs=6, name=name)

        def psB(name):
            return ps.tile([128, 512], F32, tag="B", bufs=2, name=name)

        # ---- input DMAs first: nothing in front of them in the queue ----
        xT0 = []
        for k in range(KT):
            xt = apool.tile([128, T], F32, tag="xT", bufs=8, name="xt")
            sdma(xt, x0_d[k * 128 : (k + 1) * 128, :])
            xT0.append(xt)

        # ---- layer-0 K/V and Q weights ahead of the consts in the queue ----
        wkv_sb, wq_sb, wo_sb, w1_sb, w2_sb = {}, {}, {}, {}, {}
        for k in range(KT):
            t = wpool.tile([128, 2 * E], dtw, tag="wkv", bufs=9, name="wkv_t")
            sdma(t, wkv_d[0, k * 128 : (k + 1) * 128, :])
            wkv_sb[(0, k)] = t
        for k in range(KT):
            t = wpool.tile([128, E], dtw, tag="wq", bufs=9, name="wq_t")
            sdma(t, wq_d[0, k * 128 : (k + 1) * 128, :])
            wq_sb[(0, k)] = t

        # ---- two warmup collectives: the first ~2 collectives after NEFF
        # start cost ~25us each (ncfw cold start); burn that in parallel with
        # layer-0 compute so the real layer-0 AllReduce runs at warm cost ----
        warm_sb = const.tile([128, 4], F32, tag="warm_sb")
        nc.vector.memset(warm_sb, 0.0)
        for wi in range(2):
            warm_in = dram.tile([128, 4], F32, tag="warm_in", bufs=2, name="warm_in")
            warm_out = dram.tile(
                [128, 4], F32, tag="warm_out", bufs=2,
                addr_space="Shared", name="warm_out",
            )
            nc.gpsimd.dma_start(warm_in, warm_sb)
            nc.gpsimd.collective_compute(
                "AllReduce",
                AO.add,
                ins=[warm_in.opt()],
                outs=[warm_out.opt()],
                replica_groups=[list(range(N_CORES))],
            )

        # ---- constants ----
        ones_c = const.tile([128, 1], F32, tag="ones_c")
        nc.vector.memset(ones_c, 1.0)
        ones_ch = const.tile([128, 1], BF16, tag="ones_ch")
        nc.vector.memset(ones_ch, 1.0)
        ones_r = const.tile([1, 128], BF16, tag="ones_r")
        nc.vector.memset(ones_r, 1.0 / E)
        eps_col = const.tile([128, 1], F32, tag="eps_col")
        nc.vector.memset(eps_col, EPS)
        lnfg_sb = const.tile([128, KT], F32, tag="lnfg")
        sdma(lnfg_sb, lnfg_d.rearrange("(o p) -> p o", p=128))
        lnfb_sb = const.tile([128, KT], F32, tag="lnfb")
        sdma(lnfb_sb, lnfb_d.rearrange("(o p) -> p o", p=128))
        bq_all = const.tile([128, L * KT], F32, tag="bq_all")
        sdma(bq_all, bq_d.rearrange("l (o p) -> p (l o)", p=128))
        bo_all = const.tile([128, L * KT], F32, tag="bo_all")
        sdma(bo_all, bo_d.rearrange("l (o p) -> p (l o)", p=128))
        b2_all = const.tile([128, L * KT], F32, tag="b2_all")
        sdma(b2_all, b2_d.rearrange("l (o p) -> p (l o)", p=128))
        b1_all = const.tile([128, L * FT], F32, tag="b1_all")
        sdma(b1_all, b1_d.rearrange("l (o p) -> p (l o)", p=128))
        s1_all = const.tile([128, L * FT], F32, tag="s1_all")
        sdma(s1_all, s1_d.rearrange("l (o p) -> p (l o)", p=128))
        s2_all = const.tile([128, L * KT], F32, tag="s2_all")
        sdma(s2_all, s2_d.rearrange("l (o p) -> p (l o)", p=128))
        bkv_all = const.tile([128, 2 * E], dtw, tag="bkv_all")
        ones32 = const.tile([128, 128], dtw, tag="ones32")
        nc.vector.memset(ones32, 1.0)
        for _l in range(L):
            sdma(bkv_all[32 * _l : 32 * _l + 1, :], bkv_d[_l].unsqueeze(0))

        # ---- upfront weight stream, consumption order, deep rings ----
        for l in range(n_layers):
            for k in range(KT):
                if (l, k) in wkv_sb:
                    continue
                t = wpool.tile([128, 2 * E], dtw, tag="wkv", bufs=9, name="wkv_t")
                sdma(t, wkv_d[l, k * 128 : (k + 1) * 128, :])
                wkv_sb[(l, k)] = t
            for k in range(KT):
                if (l, k) in wq_sb:
                    continue
                t = wpool.tile([128, E], dtw, tag="wq", bufs=9, name="wq_t")
                sdma(t, wq_d[l, k * 128 : (k + 1) * 128, :])
                wq_sb[(l, k)] = t
            for k in range(KT):
                t = wpool.tile([128, E], dtw, tag="wo", bufs=9, name="wo_t")
                sdma(t, wo_d[l, k * 128 : (k + 1) * 128, :])
                wo_sb[(l, k)] = t
            for tt in range(FT // 2):
                for i in range(2):
                    j = 2 * tt + i
                    t = wpool.tile([128, 3, 2, 128], FP8, tag="w1", bufs=12, name="w1_t")
                    sdma(t, w1_d[l, j])
                    w1_sb[(l, j)] = t
                t = wpool.tile([128, 2, E], FP8, tag="w2", bufs=8, name="w2_t")
                sdma(t, w2_d[l, tt])
                w2_sb[(l, tt)] = t

        def layernorm(x_tiles, out_dt, out_tag, out_bufs, gcol=None, bcol=None,
                      out_views=None):
            """(x - mu) * rsqrt(var + eps) per token; x in T-layout f32.

            Per-token sums via f32r ones-matmuls; sums broadcast to all 128
            partitions with a rank-1 ones matmul, then the whole
            mu/var/rsqrt/normalize chain runs at full 128-partition width.
            """
            sq = []
            for k in range(KT):
                sqt = apool.tile([128, T], BF16, tag="sq", bufs=2, name="sqt")
                nc.scalar.activation(sqt, x_tiles[k], AF.Square)
                sq.append(sqt)
            stat = psB("stat")
            for k in range(KT):
                nc.tensor.matmul(
                    stat[0:1, 0:T], ones_c, x_tiles[k],
                    start=(k == 0), stop=(k == KT - 1),
                )
            for k in range(KT):
                nc.tensor.matmul(
                    stat[0:1, T : 2 * T], ones_ch, sq[k],
                    start=(k == 0), stop=(k == KT - 1),
                )
            stat_sb = apool.tile([1, 2 * T], BF16, tag="stat_sb", bufs=2, name="stat_sb")
            nc.scalar.activation(stat_sb, stat[0:1, :], AF.Copy)
            # bc = [mu | E[x^2]] broadcast to all partitions (ones_r carries 1/E)
            bc = psB("bc")
            nc.tensor.matmul(bc, ones_r, stat_sb, start=True, stop=True)
            # var/sd/rs chain emitted FIRST so the t1 ops queue behind it on
            # the vector engine instead of ahead of it (critical-path order).
            mu2 = apool.tile([128, T], F32, tag="mu2", bufs=2, name="mu2")
            nc.scalar.activation(mu2, bc[:, 0:T], AF.Square)
            var = apool.tile([128, T], F32, tag="var", bufs=2, name="var")
            nc.vector.tensor_sub(var, bc[:, T : 2 * T], mu2)
            sd = apool.tile([128, T], F32, tag="sd", bufs=2, name="sd")
            nc.scalar.activation(sd, var, AF.Sqrt, bias=eps_col)
            rs = apool.tile([128, T], F32, tag="rs", bufs=2, name="rs")
            nc.vector.reciprocal_approx_fast(rs, sd)
            t1 = []
            for k in range(KT):
                t1k = apool.tile([128, T], F32, tag="lntmp", bufs=7, name="t1k")
                nc.vector.tensor_sub(t1k, x_tiles[k], bc[:, 0:T])
                t1.append(t1k)
            outs = []
            for k in range(KT):
                if out_views is not None:
                    ot = out_views[k]
                else:
                    ot = apool.tile([128, T], out_dt, tag=out_tag, bufs=out_bufs, name="lnout")
                if gcol is None:
                    nc.vector.tensor_mul(ot, t1[k], rs)
                else:
                    tmp2 = apool.tile([128, T], F32, tag="lntmp2", bufs=2, name="tmp2")
                    nc.vector.tensor_mul(tmp2, t1[k], rs)
                    nc.vector.tensor_scalar(
                        ot, tmp2,
                        gcol[:, k : k + 1], bcol[:, k : k + 1],
                        op0=AO.mult, op1=AO.add,
                    )
                outs.append(ot)
            return outs

        def layer(l, xT):
            bq_sb = bq_all[:, l * KT : (l + 1) * KT]
            bo_sb = bo_all[:, l * KT : (l + 1) * KT]
            b2_sb = b2_all[:, l * KT : (l + 1) * KT]

            # ---- LN1 ----
            hT = layernorm(xT, dtw, "hT", 7)

            # ---- K,V: stationary = hT slices, moving = Wkv (N-layout out) ----
            kv_ps = [psA(f"kv_ps_{i}") for i in range(6)]
            for k in range(KT):
                for m in range(2):
                    for n in range(3):
                        nc.tensor.matmul(
                            kv_ps[3 * m + n],
                            hT[k][:, m * 128 : (m + 1) * 128],
                            wkv_sb[(l, k)][:, n * 512 : (n + 1) * 512],
                            start=(k == 0),
                            stop=False,
                        )
            for m in range(2):
                for n in range(3):
                    nc.tensor.matmul(
                        kv_ps[3 * m + n],
                        ones32[32 * l : 32 * l + 1, :],
                        bkv_all[32 * l : 32 * l + 1, n * 512 : (n + 1) * 512],
                        start=False,
                        stop=True,
                        tile_position=(32 * l, 0),
                    )
            KV = []
            for m in range(2):
                kvt = apool.tile([128, 2 * E], dtw, tag="KV", bufs=2, name="kvt")
                for n in range(3):
                    dst = kvt[:, n * 512 : (n + 1) * 512]
                    if n == 1:
                        nc.scalar.activation(dst, kv_ps[3 * m + n], AF.Copy)
                    else:
                        nc.vector.tensor_copy(dst, kv_ps[3 * m + n])
                KV.append(kvt)

            # ---- K^T V partials (contraction over local tokens) ----
            ktv_ps = psB("ktv_ps")[:, 0 : 6 * DH]
            for j in range(6):
                for i in range(2):
                    h = 2 * j + i
                    for m in range(2):
                        nc.tensor.matmul(
                            ktv_ps[i * 64 : (i + 1) * 64, j * 64 : (j + 1) * 64],
                            KV[m][:, h * DH : (h + 1) * DH],
                            KV[m][:, E + h * DH : E + (h + 1) * DH],
                            start=(m == 0),
                            stop=(m == 1),
                            tile_position=(0, i * 64),
                        )
            ktv_sb = apool.tile([128, 6 * DH], dtw, tag="ktv_sb", bufs=2, name="ktv_sb")
            nc.scalar.activation(ktv_sb, ktv_ps, AF.Copy)

            if collective:
                cc_in = dram.tile([128, 6 * DH], dtw, tag="cc_in", bufs=2, name="cc_in")
                cc_out = dram.tile(
                    [128, 6 * DH], dtw, tag="cc_out", bufs=2,
                    addr_space="Shared", name="cc_out",
                )
                nc.gpsimd.dma_start(cc_in, ktv_sb)
                nc.gpsimd.collective_compute(
                    "AllReduce",
                    AO.add,
                    ins=[cc_in.opt()],
                    outs=[cc_out.opt()],
                    replica_groups=[list(range(N_CORES))],
                )
                ktv_w = apool.tile([128, 6 * DH], dtw, tag="ktv_w", bufs=2, name="ktv_w")
                nc.gpsimd.dma_start(ktv_w, cc_out)
            else:
                ktv_w = ktv_sb

            # ---- Q^T, emitted inside the AllReduce window (independent) ----
            QT = []
            for m in range(KT):
                qps = psB("q_ps")[:, 0:T]
                for k in range(KT):
                    nc.tensor.matmul(
                        qps,
                        wq_sb[(l, k)][:, m * 128 : (m + 1) * 128],
                        hT[k],
                        start=(k == 0),
                        stop=(k == KT - 1),
                    )
                qt = apool.tile([128, T], dtw, tag="QT", bufs=7, name="qt")
                nc.scalar.activation(qt, qps, AF.Identity, bias=bq_sb[:, m : m + 1])
                QT.append(qt)

            # ---- a^T: lhsT = KtV slice, rhs = Q^T head ----
            aT = []
            for j in range(6):
                a_ps = psB(f"a_ps_{j}")[:, 0:T]
                for i in range(2):
                    nc.tensor.matmul(
                        a_ps[i * 64 : (i + 1) * 64, :],
                        ktv_w[i * 64 : (i + 1) * 64, j * 64 : (j + 1) * 64],
                        QT[j][i * 64 : (i + 1) * 64, :],
                        start=True,
                        stop=True,
                        tile_position=(i * 64, i * 64),
                    )
                at = apool.tile([128, T], dtw, tag="aT", bufs=7, name="at")
                nc.scalar.activation(at, a_ps, AF.Copy)
                aT.append(at)

            # ---- o = a @ Wo + bo + x (residual) ----
            x2T = []
            for m in range(KT):
                ops_ = psB("o_ps")[:, 0:T]
                for k in range(KT):
                    nc.tensor.matmul(
                        ops_,
                        wo_sb[(l, k)][:, m * 128 : (m + 1) * 128],
                        aT[k],
                        start=(k == 0),
                        stop=(k == KT - 1),
                    )
                x2 = apool.tile([128, T], F32, tag="x2T", bufs=7, name="x2")
                nc.vector.scalar_tensor_tensor(
                    x2, ops_, bo_sb[:, m : m + 1], xT[m], op0=AO.add, op1=AO.add
                )
                x2T.append(x2)

            # ---- LN2: write fp8 h2 directly into the DoubleRow-interleaved tile ----
            h2p = apool.tile([128, 3, 2, T], FP8, tag="h2p", bufs=2, name="h2p")
            layernorm(
                x2T, FP8, "", 0,
                out_views=[h2p[:, k // 2, k % 2, :] for k in range(KT)],
            )

            # ---- MLP (fp8 DoubleRow): per ff-tile j, z = gelu(s1*(h2q @ W1q) + b1),
            # z pairs contract into 6 resident PSUM accumulators via W2q. ----
            m_ps = [psA(f"m_ps_{m}") for m in range(KT)]
            for tt in range(FT // 2):
                zp = apool.tile([128, 2, T], FP8, tag="zp", bufs=3, name="zp")
                for i in range(2):
                    j = 2 * tt + i
                    z_ps = psB("z_ps")[:, 0:T]
                    w1t = w1_sb[(l, j)]
                    for s in range(3):
                        nc.tensor.matmul(
                            z_ps,
                            w1t[:, s],
                            h2p[:, s],
                            start=(s == 0),
                            stop=(s == 2),
                            perf_mode=DR,
                        )
                    fi = l * FT + j
                    nc.scalar.activation(
                        zp[:, i, :], z_ps, AF.Gelu,
                        bias=b1_all[:, fi : fi + 1],
                        scale=s1_all[:, fi : fi + 1],
                    )
                w2t = w2_sb[(l, tt)]
                for m in range(KT):
                    nc.tensor.matmul(
                        m_ps[m][:, 0:T],
                        w2t[:, :, m * 128 : (m + 1) * 128],
                        zp,
                        start=(tt == 0),
                        stop=(tt == FT // 2 - 1),
                        perf_mode=DR,
                    )
            nxt = []
            for m in range(KT):
                s2col = s2_all[:, l * KT + m : l * KT + m + 1]
                tmpm = apool.tile([128, T], F32, tag="tmpm", bufs=2, name="tmpm")
                nc.scalar.activation(
                    tmpm, m_ps[m][:, 0:T], AF.Identity,
                    scale=s2col, bias=b2_sb[:, m : m + 1],
                )
                xn = apool.tile([128, T], F32, tag="xT", bufs=8, name="xn")
                nc.vector.tensor_add(xn, tmpm, x2T[m])
                nxt.append(xn)
            return nxt

        for _rep in range(reps):
            xT = xT0

            for l in range(n_layers):
                xT = layer(l, xT)

            # ---- final LN (with gain/bias) + store in T-layout ----
            fT = layernorm(xT, F32, "QT", 7, gcol=lnfg_sb, bcol=lnfb_sb)
            for k in range(KT):
                sdma(out_d[k * 128 : (k + 1) * 128, :], fT[k])

    nc.compile()
    return nc


class SpmdRunner:
    """Reusable jitted SPMD runner (modeled on bass2jax.run_bass_via_pjrt,
    without donation, so it can be invoked repeatedly)."""

    def __init__(self, nc, n_cores=N_CORES):
        bass2jax.install_neuronx_cc_hook()
        self.nc = nc
        self.n_cores = n_cores
        partition_name = nc.partition_id_tensor.name if nc.partition_id_tensor else None
        in_names, out_names, out_avals = [], [], []
        for alloc in nc.m.functions[0].allocations:
            if not isinstance(alloc, mybir.MemoryLocationSet):
                continue
            name = alloc.memorylocations[0].name
            if alloc.kind == "ExternalInput":
                if name != partition_name:
                    in_names.append(name)
            elif alloc.kind == "ExternalOutput":
                out_names.append(name)
                out_avals.append(
                    jax.core.ShapedArray(
                        tuple(alloc.tensor_shape), mybir.dt.np(alloc.dtype)
                    )
                )
        self.in_names, self.out_names, self.out_avals = in_names, out_names, out_avals
        n_params = len(in_names)
        all_in_names = list(in_names) + list(out_names)
        if partition_name is not None:
            all_in_names.append(partition_name)

        def _body(*args):
            operands = list(args)
            if partition_name is not None:
                operands.append(bass2jax.partition_id_tensor())
            outs = bass2jax._bass_exec_p.bind(
                *operands,
                out_avals=tuple(out_avals),
                in_names=tuple(all_in_names),
                out_names=tuple(out_names),
                lowering_input_output_aliases=(),
                sim_require_finite=True,
                sim_require_nnan=True,
                nc=nc,
            )
            return tuple(outs)

        devices = jax.devices()[:n_cores]
        self.mesh = Mesh(np.asarray(devices), ("core",))
        n_outs = len(out_names)
        in_specs = (PartitionSpec("core"),) * (n_params + n_outs)
        out_specs = (PartitionSpec("core"),) * n_outs
        self.fn = jax.jit(
            shard_map(
                _body,
                mesh=self.mesh,
                in_specs=in_specs,
                out_specs=out_specs,
                check_rep=False,
            ),
            keep_unused=True,
        )
        self.args = None

    def stage(self, in_maps):
        n = self.n_cores
        concat_in = [
            np.concatenate([np.asarray(in_maps[c][name]) for c in range(n)], axis=0)
            for name in self.in_names
        ]
        concat_zero = [
            np.zeros((n * a.shape[0], *a.shape[1:]), a.dtype) for a in self.out_avals
        ]
        sh = NamedSharding(self.mesh, PartitionSpec("core"))
        self.args = [jax.device_put(a, sh) for a in concat_in + concat_zero]

    def run(self):
        return self.fn(*self.args)

    def results(self, out_arrs):
        n = self.n_cores
        return [
            {
                name: np.asarray(out_arrs[i]).reshape(n, *self.out_avals[i].shape)[c]
                for i, name in enumerate(self.out_names)
            }
            for c in range(n)
        ]


def preprocess(inputs):
    """Host-side: fold LN gains into weights, shard tokens, build in_maps."""
    f = np.float32
    ie = np.asarray(inputs["inputs_embeds"], f)[0]  # [S, E]
    wpe = np.asarray(inputs["wpe"], f)[:S]
    g1 = np.asarray(inputs["ln1_g"], f)
    b1l = np.asarray(inputs["ln1_b"], f)
    g2 = np.asarray(inputs["ln2_g"], f)
    b2l = np.asarray(inputs["ln2_b"], f)
    Wq = np.asarray(inputs["Wq"], f)
    Wk = np.asarray(inputs["Wk"], f)
    Wv = np.asarray(inputs["Wv"], f)
    Wo = np.asarray(inputs["Wo"], f)
    W1 = np.asarray(inputs["W1"], f)
    W2 = np.asarray(inputs["W2"], f)
    bq = np.asarray(inputs["bq"], f)
    bk = np.asarray(inputs["bk"], f)
    bv = np.asarray(inputs["bv"], f)
    bo = np.asarray(inputs["bo"], f)
    b1 = np.asarray(inputs["b1"], f)
    b2 = np.asarray(inputs["b2"], f)

    scale = 1.0 / np.sqrt(DH)
    Wq_p = g1[:, :, None] * Wq * scale
    bq_p = (np.einsum("le,lef->lf", b1l, Wq) + bq) * scale
    Wk_p = g1[:, :, None] * Wk
    bk_p = np.einsum("le,lef->lf", b1l, Wk) + bk
    Wv_p = g1[:, :, None] * Wv
    bv_p = np.einsum("le,lef->lf", b1l, Wv) + bv
    Wkv = np.concatenate([Wk_p, Wv_p], axis=2)
    bkv = np.concatenate([bk_p, bv_p], axis=1)
    W1_p = g2[:, :, None] * W1
    b1_p = np.einsum("le,lef->lf", b2l, W1) + b1

    # fp8 per-output-channel quantization for the MLP weights
    FP8_MAX = 240.0
    fp8 = mybir.dt.np(mybir.dt.float8e4)
    s1 = np.maximum(np.abs(W1_p).max(axis=1), 1e-12) / FP8_MAX  # [L, FF]
    W1q = (W1_p / s1[:, None, :]).astype(fp8)
    s2 = np.maximum(np.abs(W2).max(axis=1), 1e-12) / FP8_MAX  # [L, E]
    W2q = (W2 / s2[:, None, :]).astype(fp8)
    # w1 packed: [L, FT, 128, 3, 2, 128]; w2 packed: [L, 12, 128, 2, E]
    W1_packed = np.ascontiguousarray(
        W1q.reshape(L, 3, 2, 128, FT, 128).transpose(0, 4, 3, 1, 2, 5)
    )
    W2_packed = np.ascontiguousarray(
        W2q.reshape(L, FT // 2, 2, 128, E).transpose(0, 1, 3, 2, 4)
    )

    cast = lambda a: np.ascontiguousarray(a).astype(ml_dtypes.bfloat16)

    watt = cast(np.concatenate([Wq_p, Wkv, Wo], axis=2))
    aux = np.concatenate([
        s1.ravel(), b1_p.ravel(), s2.ravel(), bq_p.ravel(),
        bo.ravel(), b2.ravel(),
        np.asarray(inputs["lnf_g"], f).ravel(),
        np.asarray(inputs["lnf_b"], f).ravel(),
    ]).astype(f)
    common = {
        "watt": watt,
        "w1": W1_packed,
        "w2": W2_packed,
        "aux": np.ascontiguousarray(aux),
        "bkv": cast(bkv),
    }
    x0 = ie + wpe
    maps = []
    for c in range(N_CORES):
        sl = slice(c * T, (c + 1) * T)
        maps.append({**common, "x0": np.ascontiguousarray(x0[sl].T)})
    return maps


_RUNNER = None


def _get_runner():
    global _RUNNER
    if _RUNNER is None:
        nc = build_model(reps=1)
        _RUNNER = SpmdRunner(nc)
    return _RUNNER


def kernel(**inputs):
    runner = _get_runner()
    maps = preprocess(inputs)
    runner.stage(maps)
    outs = runner.run()
    res = runner.results(outs)
    full = np.concatenate(
        [np.ascontiguousarray(res[c]["out"].T) for c in range(N_CORES)], axis=0
    )
    return full[None].astype(np.float32)


# revision 24
# speedup vs baseline: 1.0017x; 1.0017x over previous
"""Trainium2 Bass kernel for nn_NoSoftmaxGPT2Model (4-layer GPT2, no softmax).

Strategy: the missing softmax makes attention linear, so (Q K^T) V is
reassociated to Q (K^T V) -- K^T V is only [64, 64] per head. This kills the
S x S attention entirely and makes every op except that contraction
token-local. We shard the 2048-token sequence across 8 NeuronCores (256
tokens each), replicate the weights, and per layer AllReduce only the tiny
[12, 64, 64] K^T V partial sums (196 KB).

On-chip layout: activations live in SBUF transposed, [feature_part, token_free]
(T-layout). LayerNorm stats (per-token sums over features = partition
reduction) are computed with ones-vector matmuls on the PE in f32r (1 cyc/row
at free>=256), broadcast to all 128 partitions with a rank-1 ones matmul, and
the rsqrt/normalize chain runs full-width on vector+scalar. LN gains are
folded into the following weight matrices on the host.

Scheduling: all weight DMAs are emitted up-front in consumption order on one
sync queue with deep tile-pool rings, so the HBM stream runs continuously --
including through the per-layer AllReduce windows. The Q projection is emitted
between the AllReduce trigger and its first consumer so the PE has work during
the collective. PSUM is split into a 6-bank ring (KV / MLP accumulators) and a
2-bank ring (everything else), letting the MLP hold its 6 output accumulators
across all 24 ff-tiles while z-tiles rotate through 2 banks.

kernel(**inputs) takes the full unsharded inputs and returns the full
[1, 2048, 768] output.
"""

import os
from contextlib import ExitStack

import numpy as np
import ml_dtypes

import jax
from jax.sharding import Mesh, PartitionSpec, NamedSharding

import concourse.bass as bass
import concourse.bacc as bacc
import concourse.mybir as mybir
import concourse.tile as tile
from concourse.tile import add_dep_helper
from concourse import bass2jax

from jax.experimental.shard_map import shard_map

N_CORES = 8
L, S, E, H, FF = 4, 2048, 768, 12, 3072
DH = E // H  # 64
T = S // N_CORES  # 256 tokens per core
KT = E // 128  # 6 feature tiles
FT = FF // 128  # 24 ff tiles
EPS = 1e-5

F32 = mybir.dt.float32
F32R = mybir.dt.float32r
BF16 = mybir.dt.bfloat16
FP8 = mybir.dt.float8e4
DR = mybir.MatmulPerfMode.DoubleRow
AF = mybir.ActivationFunctionType
AO = mybir.AluOpType


def _r(ap):
    """f32 -> f32r view for PE ops where reduced precision is fine."""
    return ap.bitcast(F32R)


def build_model(reps=1, n_layers=L, collective=True):
    dtw = BF16
    nc = bacc.Bacc(
        "TRN2", target_bir_lowering=False, debug=False, num_devices=N_CORES
    )

    # x0 arrives host-pre-transposed to T-layout [E, T]
    x0_d = nc.dram_tensor("x0", [E, T], F32, kind="ExternalInput").ap()
    # attention weights concatenated on the last axis: [wq | wkv | wo]
    watt_d = nc.dram_tensor("watt", [L, E, 4 * E], dtw, kind="ExternalInput").ap()
    wq_d = watt_d[:, :, 0:E]
    wkv_d = watt_d[:, :, E : 3 * E]
    wo_d = watt_d[:, :, 3 * E : 4 * E]
    # w1 packed per ff-tile for DoubleRow: w1p[l,j,p,s,ko,c] = W1q[l,(2s+ko)*128+p, j*128+c]
    w1_d = nc.dram_tensor("w1", [L, FT, 128, 3, 2, 128], FP8, kind="ExternalInput").ap()
    # w2 packed per ff-pair: w2p[l,t,p,ko,c] = W2q[l,(2t+ko)*128+p, c]
    w2_d = nc.dram_tensor("w2", [L, FT // 2, 128, 2, E], FP8, kind="ExternalInput").ap()
    # small f32 aux blob: s1[L,FF] | b1[L,FF] | s2[L,E] | bq[L,E] | bo[L,E] | b2[L,E] | lnfg[E] | lnfb[E]
    AUX_SZ = 2 * L * FF + 4 * L * E + 2 * E
    aux_d = nc.dram_tensor("aux", [AUX_SZ], F32, kind="ExternalInput").ap()
    _o = [0]

    def _aux(n):
        a = aux_d[_o[0] : _o[0] + n]
        _o[0] += n
        return a

    s1_d = _aux(L * FF).rearrange("(l f) -> l f", l=L)
    b1_d = _aux(L * FF).rearrange("(l f) -> l f", l=L)
    s2_d = _aux(L * E).rearrange("(l f) -> l f", l=L)
    bq_d = _aux(L * E).rearrange("(l f) -> l f", l=L)
    bo_d = _aux(L * E).rearrange("(l f) -> l f", l=L)
    b2_d = _aux(L * E).rearrange("(l f) -> l f", l=L)
    lnfg_d = _aux(E)
    lnfb_d = _aux(E)
    bkv_d = nc.dram_tensor("bkv", [L, 2 * E], dtw, kind="ExternalInput").ap()
    # output leaves in T-layout [E, T]; host transposes back
    out_d = nc.dram_tensor("out", [E, T], F32, kind="ExternalOutput").ap()

    with tile.TileContext(nc) as tc, ExitStack() as ctx:
        const = ctx.enter_context(tc.tile_pool(name="const", bufs=1))
        wpool = ctx.enter_context(tc.tile_pool(name="wpool", bufs=1))
        apool = ctx.enter_context(tc.tile_pool(name="apool", bufs=1))
        ps = ctx.enter_context(tc.tile_pool(name="ps", bufs=1, space="PSUM"))
        dram = ctx.enter_context(tc.tile_pool(name="dram", bufs=1, space="DRAM"))

        _prev_dma = [None]

        def sdma(dst, src):
            """sync-queue DMA with forced emission-order enqueue (prevents
            scheduler-reordered slot-wait deadlocks in the shared FIFO)."""
            inst = nc.sync.dma_start(dst, src)
            if _prev_dma[0] is not None:
                add_dep_helper(inst.ins, _prev_dma[0].ins, sync=False, reason="dma order")
            _prev_dma[0] = inst
            return inst

        def psA(name):
            return ps.tile([128, 512], F32, tag="A", bufs=6, name=name)

        def psB(name):
            return ps.tile([128, 512], F32, tag="B", bufs=2, name=name)

        # ---- input DMAs first: nothing in front of them in the queue ----
        xT0 = []
        for k in range(KT):
            xt = apool.tile([128, T], F32, tag="xT", bufs=8, name="xt")
            sdma(xt, x0_d[k * 128 : (k + 1) * 128, :])
            xT0.append(xt)

        # ---- layer-0 K/V and Q weights ahead of the consts in the queue ----
        wkv_sb, wq_sb, wo_sb, w1_sb, w2_sb = {}, {}, {}, {}, {}
        for k in range(KT):
            t = wpool.tile([128, 2 * E], dtw, tag="wkv", bufs=9, name="wkv_t")
            sdma(t, wkv_d[0, k * 128 : (k + 1) * 128, :])
            wkv_sb[(0, k)] = t
        for k in range(KT):
            t = wpool.tile([128, E], dtw, tag="wq", bufs=9, name="wq_t")
            sdma(t, wq_d[0, k * 128 : (k + 1) * 128, :])
            wq_sb[(0, k)] = t

        # ---- two warmup collectives: the first ~2 collectives after NEFF
        # start cost ~25us each (ncfw cold start); burn that in parallel with
        # layer-0 compute so the real layer-0 AllReduce runs at warm cost ----
        warm_sb = const.tile([128, 4], F32, tag="warm_sb")
        nc.vector.memset(warm_sb, 0.0)
        for wi in range(2):
            warm_in = dram.tile([128, 4], F32, tag="warm_in", bufs=2, name="warm_in")
            warm_out = dram.tile(
                [128, 4], F32, tag="warm_out", bufs=2,
                addr_space="Shared", name="warm_out",
            )
            nc.gpsimd.dma_start(warm_in, warm_sb)
            nc.gpsimd.collective_compute(
                "AllReduce",
                AO.add,
                ins=[warm_in.opt()],
                outs=[warm_out.opt()],
                replica_groups=[list(range(N_CORES))],
            )

        # ---- constants ----
        ones_c = const.tile([128, 1], F32, tag="ones_c")
        nc.vector.memset(ones_c, 1.0)
        ones_ch = const.tile([128, 1], BF16, tag="ones_ch")
        nc.vector.memset(ones_ch, 1.0)
        ones_r = const.tile([1, 128], BF16, tag="ones_r")
        nc.vector.memset(ones_r, 1.0 / E)
        eps_col = const.tile([128, 1], F32, tag="eps_col")
        nc.vector.memset(eps_col, EPS)
        lnfg_sb = const.tile([128, KT], F32, tag="lnfg")
        sdma(lnfg_sb, lnfg_d.rearrange("(o p) -> p o", p=128))
        lnfb_sb = const.tile([128, KT], F32, tag="lnfb")
        sdma(lnfb_sb, lnfb_d.rearrange("(o p) -> p o", p=128))
        bq_all = const.tile([128, L * KT], F32, tag="bq_all")
        sdma(bq_all, bq_d.rearrange("l (o p) -> p (l o)", p=128))
        bo_all = const.tile([128, L * KT], F32, tag="bo_all")
        sdma(bo_all, bo_d.rearrange("l (o p) -> p (l o)", p=128))
        b2_all = const.tile([128, L * KT], F32, tag="b2_all")
        sdma(b2_all, b2_d.rearrange("l (o p) -> p (l o)", p=128))
        b1_all = const.tile([128, L * FT], F32, tag="b1_all")
        sdma(b1_all, b1_d.rearrange("l (o p) -> p (l o)", p=128))
        s1_all = const.tile([128, L * FT], F32, tag="s1_all")
        sdma(s1_all, s1_d.rearrange("l (o p) -> p (l o)", p=128))
        s2_all = const.tile([128, L * KT], F32, tag="s2_all")
        sdma(s2_all, s2_d.rearrange("l (o p) -> p (l o)", p=128))
        bkv_all = const.tile([128, 2 * E], dtw, tag="bkv_all")
        ones32 = const.tile([128, 128], dtw, tag="ones32")
        nc.vector.memset(ones32, 1.0)
        for _l in range(L):
            sdma(bkv_all[32 * _l : 32 * _l + 1, :], bkv_d[_l].unsqueeze(0))

        # ---- upfront weight stream, consumption order, deep rings ----
        for l in range(n_layers):
            for k in range(KT):
                if (l, k) in wkv_sb:
                    continue
                t = wpool.tile([128, 2 * E], dtw, tag="wkv", bufs=9, name="wkv_t")
                sdma(t, wkv_d[l, k * 128 : (k + 1) * 128, :])
                wkv_sb[(l, k)] = t
            for k in range(KT):
                if (l, k) in wq_sb:
                    continue
                t = wpool.tile([128, E], dtw, tag="wq", bufs=9, name="wq_t")
                sdma(t, wq_d[l, k * 128 : (k + 1) * 128, :])
                wq_sb[(l, k)] = t
            for k in range(KT):
                t = wpool.tile([128, E], dtw, tag="wo", bufs=9, name="wo_t")
                sdma(t, wo_d[l, k * 128 : (k + 1) * 128, :])
                wo_sb[(l, k)] = t
            for tt in range(FT // 2):
                for i in range(2):
                    j = 2 * tt + i
                    t = wpool.tile([128, 3, 2, 128], FP8, tag="w1", bufs=12, name="w1_t")
                    sdma(t, w1_d[l, j])
                    w1_sb[(l, j)] = t
                t = wpool.tile([128, 2, E], FP8, tag="w2", bufs=8, name="w2_t")
                sdma(t, w2_d[l, tt])
                w2_sb[(l, tt)] = t

        def layernorm(x_tiles, out_dt, out_tag, out_bufs, gcol=None, bcol=None,
                      out_views=None):
            """(x - mu) * rsqrt(var + eps) per token; x in T-layout f32.

            Per-token sums via f32r ones-matmuls; sums broadcast to all 128
            partitions with a rank-1 ones matmul, then the whole
            mu/var/rsqrt/normalize chain runs at full 128-partition width.
            """
            sq = []
            for k in range(KT):
                sqt = apool.tile([128, T], BF16, tag="sq", bufs=2, name="sqt")
                nc.scalar.activation(sqt, x_tiles[k], AF.Square)
                sq.append(sqt)
            stat = psB("stat")
            for k in range(KT):
                nc.tensor.matmul(
                    stat[0:1, 0:T], ones_c, x_tiles[k],
                    start=(k == 0), stop=(k == KT - 1),
                )
            for k in range(KT):
                nc.tensor.matmul(
                    stat[0:1, T : 2 * T], ones_ch, sq[k],
                    start=(k == 0), stop=(k == KT - 1),
                )
            stat_sb = apool.tile([1, 2 * T], BF16, tag="stat_sb", bufs=2, name="stat_sb")
            nc.scalar.activation(stat_sb, stat[0:1, :], AF.Copy)
            # bc = [mu | E[x^2]] broadcast to all partitions (ones_r carries 1/E)
            bc = psB("bc")
            nc.tensor.matmul(bc, ones_r, stat_sb, start=True, stop=True)
            # var/sd/rs chain emitted FIRST so the t1 ops queue behind it on
            # the vector engine instead of ahead of it (critical-path order).
            mu2 = apool.tile([128, T], F32, tag="mu2", bufs=2, name="mu2")
            nc.scalar.activation(mu2, bc[:, 0:T], AF.Square)
            var = apool.tile([128, T], F32, tag="var", bufs=2, name="var")
            nc.vector.tensor_sub(var, bc[:, T : 2 * T], mu2)
            sd = apool.tile([128, T], F32, tag="sd", bufs=2, name="sd")
            nc.scalar.activation(sd, var, AF.Sqrt, bias=eps_col)
            rs = apool.tile([128, T], F32, tag="rs", bufs=2, name="rs")
            nc.vector.reciprocal_approx_fast(rs, sd)
            t1 = []
            for k in range(KT):
                t1k = apool.tile([128, T], F32, tag="lntmp", bufs=7, name="t1k")
                nc.vector.tensor_sub(t1k, x_tiles[k], bc[:, 0:T])
                t1.append(t1k)
            outs = []
            for k in range(KT):
                if out_views is not None:
                    ot = out_views[k]
                else:
                    ot = apool.tile([128, T], out_dt, tag=out_tag, bufs=out_bufs, name="lnout")
                if gcol is None:
                    nc.vector.tensor_mul(ot, t1[k], rs)
                else:
                    tmp2 = apool.tile([128, T], F32, tag="lntmp2", bufs=2, name="tmp2")
                    nc.vector.tensor_mul(tmp2, t1[k], rs)
                    nc.vector.tensor_scalar(
                        ot, tmp2,
                        gcol[:, k : k + 1], bcol[:, k : k + 1],
                        op0=AO.mult, op1=AO.add,
                    )
                outs.append(ot)
            return outs

        def layer(l, xT):
            bq_sb = bq_all[:, l * KT : (l + 1) * KT]
            bo_sb = bo_all[:, l * KT : (l + 1) * KT]
            b2_sb = b2_all[:, l * KT : (l + 1) * KT]

            # ---- LN1 ----
            hT = layernorm(xT, dtw, "hT", 7)

            # ---- K,V: stationary = hT slices, moving = Wkv (N-layout out) ----
            kv_ps = [psA(f"kv_ps_{i}") for i in range(6)]
            for k in range(KT):
                for m in range(2):
                    for n in range(3):
                        nc.tensor.matmul(
                            kv_ps[3 * m + n],
                            hT[k][:, m * 128 : (m + 1) * 128],
                            wkv_sb[(l, k)][:, n * 512 : (n + 1) * 512],
                            start=(k == 0),
                            stop=False,
                        )
            for m in range(2):
                for n in range(3):
                    nc.tensor.matmul(
                        kv_ps[3 * m + n],
                        ones32[32 * l : 32 * l + 1, :],
                        bkv_all[32 * l : 32 * l + 1, n * 512 : (n + 1) * 512],
                        start=False,
                        stop=True,
                        tile_position=(32 * l, 0),
                    )
            KV = []
            for m in range(2):
                kvt = apool.tile([128, 2 * E], dtw, tag="KV", bufs=2, name="kvt")
                for n in range(3):
                    dst = kvt[:, n * 512 : (n + 1) * 512]
                    if n == 1:
                        nc.scalar.activation(dst, kv_ps[3 * m + n], AF.Copy)
                    else:
                        nc.vector.tensor_copy(dst, kv_ps[3 * m + n])
                KV.append(kvt)

            # ---- K^T V partials (contraction over local tokens) ----
            ktv_ps = psB("ktv_ps")[:, 0 : 6 * DH]
            for j in range(6):
                for i in range(2):
                    h = 2 * j + i
                    for m in range(2):
                        nc.tensor.matmul(
                            ktv_ps[i * 64 : (i + 1) * 64, j * 64 : (j + 1) * 64],
                            KV[m][:, h * DH : (h + 1) * DH],
                            KV[m][:, E + h * DH : E + (h + 1) * DH],
                            start=(m == 0),
                            stop=(m == 1),
                            tile_position=(0, i * 64),
                        )
            ktv_sb = apool.tile([128, 6 * DH], dtw, tag="ktv_sb", bufs=2, name="ktv_sb")
            nc.scalar.activation(ktv_sb, ktv_ps, AF.Copy)

            if collective:
                cc_in = dram.tile([128, 6 * DH], dtw, tag="cc_in", bufs=2, name="cc_in")
                cc_out = dram.tile(
                    [128, 6 * DH], dtw, tag="cc_out", bufs=2,
                    addr_space="Shared", name="cc_out",
                )
                nc.gpsimd.dma_start(cc_in, ktv_sb)
                nc.gpsimd.collective_compute(
                    "AllReduce",
                    AO.add,
                    ins=[cc_in.opt()],
                    outs=[cc_out.opt()],
                    replica_groups=[list(range(N_CORES))],
                )
                ktv_w = apool.tile([128, 6 * DH], dtw, tag="ktv_w", bufs=2, name="ktv_w")
                nc.gpsimd.dma_start(ktv_w, cc_out)
            else:
                ktv_w = ktv_sb

            # ---- Q^T, emitted inside the AllReduce window (independent) ----
            QT = []
            for m in range(KT):
                qps = psB("q_ps")[:, 0:T]
                for k in range(KT):
                    nc.tensor.matmul(
                        qps,
                        wq_sb[(l, k)][:, m * 128 : (m + 1) * 128],
                        hT[k],
                        start=(k == 0),
                        stop=(k == KT - 1),
                    )
                qt = apool.tile([128, T], dtw, tag="QT", bufs=7, name="qt")
                nc.scalar.activation(qt, qps, AF.Identity, bias=bq_sb[:, m : m + 1])
                QT.append(qt)

            # ---- a^T: lhsT = KtV slice, rhs = Q^T head ----
            aT = []
            for j in range(6):
                a_ps = psB(f"a_ps_{j}")[:, 0:T]
                for i in range(2):
                    nc.tensor.matmul(
                        a_ps[i * 64 : (i + 1) * 64, :],
                        ktv_w[i * 64 : (i + 1) * 64, j * 64 : (j + 1) * 64],
                        QT[j][i * 64 : (i + 1) * 64, :],
                        start=True,
                        stop=True,
                        tile_position=(i * 64, i * 64),
                    )
                at = apool.tile([128, T], dtw, tag="aT", bufs=7, name="at")
                nc.scalar.activation(at, a_ps, AF.Copy)
                aT.append(at)

            # ---- o = a @ Wo + bo + x (residual) ----
            x2T = []
            for m in range(KT):
                ops_ = psB("o_ps")[:, 0:T]
                for k in range(KT):
                    nc.tensor.matmul(
                        ops_,
                        wo_sb[(l, k)][:, m * 128 : (m + 1) * 128],
                        aT[k],
                        start=(k == 0),
                        stop=(k == KT - 1),
                    )
                x2 = apool.tile([128, T], F32, tag="x2T", bufs=7, name="x2")
                nc.vector.scalar_tensor_tensor(
                    x2, ops_, bo_sb[:, m : m + 1], xT[m], op0=AO.add, op1=AO.add
                )
                x2T.append(x2)

            # ---- LN2: write fp8 h2 directly into the DoubleRow-interleaved tile ----
            h2p = apool.tile([128, 3, 2, T], FP8, tag="h2p", bufs=2, name="h2p")
            layernorm(
                x2T, FP8, "", 0,
                out_views=[h2p[:, k // 2, k % 2, :] for k in range(KT)],
            )

            # ---- MLP (fp8 DoubleRow): per ff-tile j, z = gelu(s1*(h2q @ W1q) + b1),
            # z pairs contract into 6 resident PSUM accumulators via W2q. ----
            m_ps = [psA(f"m_ps_{m}") for m in range(KT)]
            for tt in range(FT // 2):
                zp = apool.tile([128, 2, T], FP8, tag="zp", bufs=3, name="zp")
                for i in range(2):
                    j = 2 * tt + i
                    z_ps = psB("z_ps")[:, 0:T]
                    w1t = w1_sb[(l, j)]
                    for s in range(3):
                        nc.tensor.matmul(
                            z_ps,
                            w1t[:, s],
                            h2p[:, s],
                            start=(s == 0),
                            stop=(s == 2),
                            perf_mode=DR,
                        )
                    fi = l * FT + j
                    nc.scalar.activation(
                        zp[:, i, :], z_ps, AF.Gelu,
                        bias=b1_all[:, fi : fi + 1],
                        scale=s1_all[:, fi : fi + 1],
                    )
                w2t = w2_sb[(l, tt)]
                for m in range(KT):
                    nc.tensor.matmul(
                        m_ps[m][:, 0:T],
                        w2t[:, :, m * 128 : (m + 1) * 128],
                        zp,
                        start=(tt == 0),
                        stop=(tt == FT // 2 - 1),
                        perf_mode=DR,
                    )
            nxt = []
            for m in range(KT):
                s2col = s2_all[:, l * KT + m : l * KT + m + 1]
                tmpm = apool.tile([128, T], F32, tag="tmpm", bufs=2, name="tmpm")
                nc.scalar.activation(
                    tmpm, m_ps[m][:, 0:T], AF.Identity,
                    scale=s2col, bias=b2_sb[:, m : m + 1],
                )
                xn = apool.tile([128, T], F32, tag="xT", bufs=8, name="xn")
                nc.vector.tensor_add(xn, tmpm, x2T[m])
                nxt.append(xn)
            return nxt

        for _rep in range(reps):
            xT = xT0

            for l in range(n_layers):
                xT = layer(l, xT)

            # ---- final LN (with gain/bias) + store in T-layout ----
            fT = layernorm(xT, F32, "QT", 7, gcol=lnfg_sb, bcol=lnfb_sb)
            for k in range(KT):
                sdma(out_d[k * 128 : (k + 1) * 128, :], fT[k])

    nc.compile()
    return nc


class SpmdRunner:
    """Reusable jitted SPMD runner (modeled on bass2jax.run_bass_via_pjrt,
    without donation, so it can be invoked repeatedly)."""

    def __init__(self, nc, n_cores=N_CORES):
        bass2jax.install_neuronx_cc_hook()
        self.nc = nc
        self.n_cores = n_cores
        partition_name = nc.partition_id_tensor.name if nc.partition_id_tensor else None
        in_names, out_names, out_avals = [], [], []
        for alloc in nc.m.functions[0].allocations:
            if not isinstance(alloc, mybir.MemoryLocationSet):
                continue
            name = alloc.memorylocations[0].name
            if alloc.kind == "ExternalInput":
                if name != partition_name:
                    in_names.append(name)
            elif alloc.kind == "ExternalOutput":
                out_names.append(name)
                out_avals.append(
                    jax.core.ShapedArray(
                        tuple(alloc.tensor_shape), mybir.dt.np(alloc.dtype)
                    )
                )
        self.in_names, self.out_names, self.out_avals = in_names, out_names, out_avals
        n_params = len(in_names)
        all_in_names = list(in_names) + list(out_names)
        if partition_name is not None:
            all_in_names.append(partition_name)

        def _body(*args):
            operands = list(args)
            if partition_name is not None:
                operands.append(bass2jax.partition_id_tensor())
            outs = bass2jax._bass_exec_p.bind(
                *operands,
                out_avals=tuple(out_avals),
                in_names=tuple(all_in_names),
                out_names=tuple(out_names),
                lowering_input_output_aliases=(),
                sim_require_finite=True,
                sim_require_nnan=True,
                nc=nc,
            )
            return tuple(outs)

        devices = jax.devices()[:n_cores]
        self.mesh = Mesh(np.asarray(devices), ("core",))
        n_outs = len(out_names)
        in_specs = (PartitionSpec("core"),) * (n_params + n_outs)
        out_specs = (PartitionSpec("core"),) * n_outs
        self.fn = jax.jit(
            shard_map(
                _body,
                mesh=self.mesh,
                in_specs=in_specs,
                out_specs=out_specs,
                check_rep=False,
            ),
            keep_unused=True,
        )
        self.args = None

    def stage(self, in_maps):
        n = self.n_cores
        concat_in = [
            np.concatenate([np.asarray(in_maps[c][name]) for c in range(n)], axis=0)
            for name in self.in_names
        ]
        concat_zero = [
            np.zeros((n * a.shape[0], *a.shape[1:]), a.dtype) for a in self.out_avals
        ]
        sh = NamedSharding(self.mesh, PartitionSpec("core"))
        self.args = [jax.device_put(a, sh) for a in concat_in + concat_zero]

    def run(self):
        return self.fn(*self.args)

    def results(self, out_arrs):
        n = self.n_cores
        return [
            {
                name: np.asarray(out_arrs[i]).reshape(n, *self.out_avals[i].shape)[c]
                for i, name in enumerate(self.out_names)
            }
            for c in range(n)
        ]


def preprocess(inputs):
    """Host-side: fold LN gains into weights, shard tokens, build in_maps."""
    f = np.float32
    ie = np.asarray(inputs["inputs_embeds"], f)[0]  # [S, E]
    wpe = np.asarray(inputs["wpe"], f)[:S]
    g1 = np.asarray(inputs["ln1_g"], f)
    b1l = np.asarray(inputs["ln1_b"], f)
    g2 = np.asarray(inputs["ln2_g"], f)
    b2l = np.asarray(inputs["ln2_b"], f)
    Wq = np.asarray(inputs["Wq"], f)
    Wk = np.asarray(inputs["Wk"], f)
    Wv = np.asarray(inputs["Wv"], f)
    Wo = np.asarray(inputs["Wo"], f)
    W1 = np.asarray(inputs["W1"], f)
    W2 = np.asarray(inputs["W2"], f)
    bq = np.asarray(inputs["bq"], f)
    bk = np.asarray(inputs["bk"], f)
    bv = np.asarray(inputs["bv"], f)
    bo = np.asarray(inputs["bo"], f)
    b1 = np.asarray(inputs["b1"], f)
    b2 = np.asarray(inputs["b2"], f)

    scale = 1.0 / np.sqrt(DH)
    Wq_p = g1[:, :, None] * Wq * scale
    bq_p = (np.einsum("le,lef->lf", b1l, Wq) + bq) * scale
    Wk_p = g1[:, :, None] * Wk
    bk_p = np.einsum("le,lef->lf", b1l, Wk) + bk
    Wv_p = g1[:, :, None] * Wv
    bv_p = np.einsum("le,lef->lf", b1l, Wv) + bv
    Wkv = np.concatenate([Wk_p, Wv_p], axis=2)
    bkv = np.concatenate([bk_p, bv_p], axis=1)
    W1_p = g2[:, :, None] * W1
    b1_p = np.einsum("le,lef->lf", b2l, W1) + b1

    # fp8 per-output-channel quantization for the MLP weights
    FP8_MAX = 240.0
    fp8 = mybir.dt.np(mybir.dt.float8e4)
    s1 = np.maximum(np.abs(W1_p).max(axis=1), 1e-12) / FP8_MAX  # [L, FF]
    W1q = (W1_p / s1[:, None, :]).astype(fp8)
    s2 = np.maximum(np.abs(W2).max(axis=1), 1e-12) / FP8_MAX  # [L, E]
    W2q = (W2 / s2[:, None, :]).astype(fp8)
    # w1 packed: [L, FT, 128, 3, 2, 128]; w2 packed: [L, 12, 128, 2, E]
    W1_packed = np.ascontiguousarray(
        W1q.reshape(L, 3, 2, 128, FT, 128).transpose(0, 4, 3, 1, 2, 5)
    )
    W2_packed = np.ascontiguousarray(
        W2q.reshape(L, FT // 2, 2, 128, E).transpose(0, 1, 3, 2, 4)
    )

    cast = lambda a: np.ascontiguousarray(a).astype(ml_dtypes.bfloat16)

    watt = cast(np.concatenate([Wq_p, Wkv, Wo], axis=2))
    aux = np.concatenate([
        s1.ravel(), b1_p.ravel(), s2.ravel(), bq_p.ravel(),
        bo.ravel(), b2.ravel(),
        np.asarray(inputs["lnf_g"], f).ravel(),
        np.asarray(inputs["lnf_b"], f).ravel(),
    ]).astype(f)
    common = {
        "watt": watt,
        "w1": W1_packed,
        "w2": W2_packed,
        "aux": np.ascontiguousarray(aux),
        "bkv": cast(bkv),
    }
    x0 = ie + wpe
    maps = []
    for c in range(N_CORES):
        sl = slice(c * T, (c + 1) * T)
        maps.append({**common, "x0": np.ascontiguousarray(x0[sl].T)})
    return maps


_RUNNER = None


def _get_runner():
    global _RUNNER
    if _RUNNER is None:
        nc = build_model(reps=1)
        _RUNNER = SpmdRunner(nc)
    return _RUNNER


def kernel(**inputs):
    runner = _get_runner()
    maps = preprocess(inputs)
    runner.stage(maps)
    outs = runner.run()
    res = runner.results(outs)
    full = np.concatenate(
        [np.ascontiguousarray(res[c]["out"].T) for c in range(N_CORES)], axis=0
    )
    return full[None].astype(np.float32)
